# revision 1
# baseline (speedup 1.0000x reference)
"""Nystrom attention Trainium2 kernel (fused landmark formulation).

Sharding: 8 cores = 4 batches x 2 head-groups (4 heads each). Each core
computes its (batch, head-group) slice; the host sums the two bf16 partial
output projections per batch (in f32) and adds bo.

Algebra (per head h, SCALE = HEAD_DIM**-0.25, q = x@Wq + bq etc.):
  x_land   = segment means of x (host; linear pooling of the input)
  q_landT  = Wq^T-contract(x_landT*SCALE) + bq*SCALE   [(h,d), L] on device
  logits1  = x @ M1T + bq.k_land         M1T = Wq-contract(k_landT)*SCALE
  logits3  = (x @ M3T)^T-ish             (kernel_3's bias is constant along
                                          its softmax axis; drops out exactly)
  K2       = softmax(q_landT^T k_landT) per head; invK2 via Newton-Schulz.
  v16      = x @ (Wv*16)   (bv folds into t1n; Wo/16 compensates the 16)
  t1       = [v16|1]^T @ exp(logits3) -> t1n = rows/rowsum + bv*16
  m        = ((invK2 @ t1n) @ (Wo/16))^T-chain => m_sb [(h,L), E]
  out      = (e1 / rowsum_head(e1)) @ m,   e1 = exp(logits1)

Big matmuls are fp8e4 DoubleRow (hi/lo compensated for v; logits are tiny
(~0.1 rms) so single fp8 is safe there). Newton-Schulz runs bf16 for
iterations 0-4 and f32 for the final iteration (last-iter precision
dominates the result), with its stages interleaved into phase A.
"""

import os
import numpy as np
import ml_dtypes

import concourse.bass as bass
import concourse.tile as tile
from concourse import bacc, mybir
from concourse.bass_utils import run_bass_kernel_spmd

BF16 = mybir.dt.bfloat16
F32 = mybir.dt.float32
FP8 = mybir.dt.float8e4
AF = mybir.ActivationFunctionType
AX = mybir.AxisListType
OP = mybir.AluOpType
PM = mybir.MatmulPerfMode

S = 8192        # sequence length
E = 512         # embedding dim
D = 64          # head dim
L = 64          # landmarks
NHG = 4         # heads per core (head group)
N_ITER = 6
SCALE = 1.0 / np.sqrt(np.sqrt(D))
M8 = 64.0       # fp8 prescale on M1T/M3T (undone by exp scale)
NSPLIT = 4      # x8/xlo DMA pipelining splits along S
USE_DR = os.environ.get("K_DR", "1") == "1"
USE_GPSIMD = os.environ.get("K_GP", "1") == "1"

_CACHED_NC = None
_TILES = {}


def _build():
    nc = bacc.Bacc("TRN2", target_bir_lowering=False, debug=False, num_devices=8)

    dram = {}
    for name, shape, dt in [
        ("x8", [E, S], FP8),
        ("xlo", [E, S], FP8),
        ("blob0", [E, 576], BF16),
        ("blob1", [256, 2 * E + 2], BF16),
        ("wv16h", [E, 256], FP8),
        ("wv16l", [E, 256], FP8),
        ("wo16", [256, E], BF16),

        ("bv16b", [64, NHG, L], F32),
        ("nsc2", [64, 3, 256], F32),
        ("idf32", [128, 128], F32),
        ("blk1", [128, 128], BF16),
        ("onesr", [1, 128], F32),
    ]:
        dram[name] = nc.dram_tensor(name, shape, dt, kind="ExternalInput").ap()
    out_d = nc.dram_tensor("out", [S, E], BF16, kind="ExternalOutput").ap()

    with tile.TileContext(nc) as tc:
        _emit(nc, tc, dram, out_d)
    nc.compile()
    return nc


def _emit(nc, tc, dram, out_d):
    SP = S // NSPLIT
    with (
        tc.tile_pool(name="const", bufs=1) as const,
        tc.tile_pool(name="big", bufs=1) as big,
        tc.tile_pool(name="small", bufs=2) as small,
    ):
        def load(name, shape, dt, pat=None, **kw):
            t = const.tile(shape, dt, tag=name)
            src = dram[name]
            if pat is not None:
                src = src.rearrange(pat, **kw)
            nc.sync.dma_start(t[:], src)
            return t

        # small consts first so phase 0 can begin immediately
        blob0 = load("blob0", [128, 4, 576], BF16, "(ko p) m -> p ko m",
                     p=128)
        xlT = blob0[:, :, 0:64]
        wq = blob0[:, :, 64:320]
        wk = blob0[:, :, 320:576]
        blob1 = load("blob1", [64, NHG, 2 * E + 2], BF16,
                     "(h p) m -> p h m", p=64)
        wqT = blob1[:, :, 0:E]
        wkT = blob1[:, :, E:2 * E]
        bqs = blob1[:, :, 2 * E:2 * E + 1]
        bks = blob1[:, :, 2 * E + 1:2 * E + 2]
        idf32 = load("idf32", [128, 128], F32)
        onesr = load("onesr", [1, 128], F32)
        wv16h = load("wv16h", [128, 4, 256], FP8, "(ko p) m -> p ko m", p=128)
        wv16l = load("wv16l", [128, 4, 256], FP8, "(ko p) m -> p ko m", p=128)

        # big inputs, split along S; first split lands before the consts that
        # are only needed later (N-S constants, phase A'/B weights) so the
        # streaming phase can start as early as possible.
        def load_split(i):
            ssl = slice(i * SP, (i + 1) * SP)
            t8 = big.tile([128, 4, SP], FP8, tag=f"x8_{i}")
            nc.sync.dma_start(
                t8[:], dram["x8"][:, ssl].rearrange("(ko p) s -> p ko s", p=128))
            tlo = big.tile([128, 4, SP], FP8, tag=f"xlo_{i}")
            nc.sync.dma_start(
                tlo[:], dram["xlo"][:, ssl].rearrange("(ko p) s -> p ko s", p=128))
            return t8, tlo

        x8s, xlos = [], []
        t8, tlo = load_split(0)
        x8s.append(t8)
        xlos.append(tlo)
        nsc2 = load("nsc2", [64, 3, 256], F32)
        bv16b = load("bv16b", [64, NHG, L], F32)
        blk1 = load("blk1", [128, 128], BF16)
        wo16 = load("wo16", [64, NHG, E], BF16, "(h p) m -> p h m", p=64)
        for i in range(1, NSPLIT):
            t8, tlo = load_split(i)
            x8s.append(t8)
            xlos.append(tlo)

        # persistent activations / results
        vsb = big.tile([128, 64, NHG, 65], BF16, tag="v")  # (s | c4, head, d+1)
        nc.vector.memset(vsb[:, :, :, 64:65], 1.0)
        m_sb = const.tile([128, 2, E], BF16, tag="m")      # out-proj matrix
        bias1 = const.tile([128, 2, 1], F32, tag="bias1")  # exp bias per (h,L)

        qlT = const.tile([64, NHG, L], BF16, tag="qlT")  # (d part, head, L)
        klT = const.tile([64, NHG, L], BF16, tag="klT")
        M1T8 = const.tile([128, 4, 256], FP8, tag="M1T8")  # (E part, hL free)
        M3T8 = const.tile([128, 4, 256], FP8, tag="M3T8")

        if os.environ.get("K_NOP0") == "1":
            with tc.tile_pool(name="zz3", bufs=1) as zz:
                z = zz.tile([128, E], BF16, tag="z")
                nc.vector.memset(z[:], 0.0)
                for c128 in range(64):
                    nc.sync.dma_start(out_d[bass.ts(c128, 128), :], z[:])
            return
        # ============ Phase 0: landmark branch + N-S inverse ============
        global _TILES
        _TILES = {"vsb": vsb, "m_sb": m_sb, "qlT": qlT, "klT": klT,
                  "M1T8": M1T8, "M3T8": M3T8, "bias1": bias1}
        W_holder = {}
        import contextlib
        ns_stages = []
        ns_es = contextlib.ExitStack()
        ps0w = ns_es.enter_context(tc.tile_pool(name="ps0w", bufs=3,
                                                space="PSUM"))
        nsp = ns_es.enter_context(tc.tile_pool(name="nsp", bufs=2))
        with tc.tile_pool(name="ps0", bufs=4, space="PSUM") as ps0:
            def p0(name):
                # one shared psum shape; 128-col (512B) slots per head, all
                # operands and outputs at base partition 0: runtime crashes
                # were traced to matmuls with base-64 operands.
                return ps0.tile([128, 512], F32, tag="ps0", name=name)

            # q_landT/k_landT: [d, h, L] = Wq_h-contract(x_landT) + bias
            for i, (dst, w, b) in enumerate(((qlT, wq, bqs), (klT, wk, bks))):
                p = p0(f"pl{i}")
                for h in range(NHG):
                    for ko in range(4):
                        nc.tensor.matmul(p[0:64, h * 128:h * 128 + L],
                                         lhsT=w[:, ko, bass.ts(h, 64)],
                                         rhs=xlT[:, ko, :],
                                         start=(ko == 0), stop=(ko == 3),
                                         skip_group_check=True)
                for h in range(NHG):
                    nc.scalar.activation(dst[:, h, :],
                                         p[0:64, h * 128:h * 128 + L],
                                         AF.Identity, bias=b[:, h, :])

            P0LVL = int(os.environ.get("K_P0LVL", "9"))
            # M1T = Wq-contract(k_landT)*SCALE*M8 (fp8), M3T symmetric.
            for im, (dstM, wT, landT) in enumerate(
                    ((M1T8, wqT, klT), (M3T8, wkT, qlT)) if P0LVL >= 2 else ()):
                for ko in range(4):
                    p = p0(f"pM{im}{ko}")
                    for h in range(NHG):
                        nc.tensor.matmul(
                            p[:, h * 128:h * 128 + 64],
                            lhsT=wT[:, h, bass.ts(ko, 128)],
                            rhs=landT[:, h, :],
                            start=True, stop=True, skip_group_check=True)
                    nc.vector.tensor_scalar_mul(
                        dstM[:, ko, :],
                        p[:].rearrange("p (h s) -> p h s", s=128)[:, :, 0:64],
                        SCALE * M8)

            if P0LVL < 6:
                W_holder["W"] = None
            if P0LVL >= 3:
                # bias1[(h,l)] = sum_d bqs[(h,d)] k_landT[(h,d), l]
                pb2 = p0("pb2")
                for h in range(NHG):
                    nc.tensor.matmul(pb2[0:64, h * 128:h * 128 + 1],
                                     lhsT=klT[:, h, :],
                                     rhs=bqs[:, h, :], start=True, stop=True,
                                     skip_group_check=True)
                for h in range(NHG):
                    t, psl = h // 2, bass.ts(h % 2, 64)
                    nc.vector.tensor_copy(bias1[psl, t, :],
                                          pb2[0:64, h * 128:h * 128 + 1])

            if P0LVL >= 4:
                # K2 softmax + N-S init run as deferred stages too (their
                # serial chains would otherwise block phase A in the
                # in-order PE queue).
                NSX = {}

                def st_K2():
                    K2 = nsp.tile([64, NHG, L], F32, tag="K2", name="K2")
                    K2T = nsp.tile([64, NHG, L], F32, tag="K2T", name="K2T")
                    k2e = small.tile([64, NHG, L], F32, tag="k2e")
                    rs = small.tile([64, NHG, 1], F32, tag="k2rs")
                    ri = small.tile([64, NHG, 1], F32, tag="k2ri")
                    pk2 = ps0w.tile([64, 512], F32, tag="ps0w", name="pk2")
                    for h in range(NHG):
                        nc.tensor.matmul(pk2[0:64, h * 128:h * 128 + L],
                                         lhsT=qlT[:, h, :],
                                         rhs=klT[:, h, :], start=True,
                                         stop=True, skip_group_check=True)
                    for h in range(NHG):
                        nc.scalar.activation(k2e[:, h, :],
                                             pk2[0:64, h * 128:h * 128 + L],
                                             AF.Exp, accum_out=rs[:, h, :])
                    nc.vector.reciprocal(ri[:], rs[:])
                    for h in range(NHG):
                        nc.vector.tensor_scalar_mul(K2[:, h, :], k2e[:, h, :],
                                                    ri[:, h, :])
                    pt = ps0w.tile([64, 512], F32, tag="ps0w", name="pk2t")
                    for h in range(NHG):
                        nc.tensor.transpose(pt[0:64, h * 128:h * 128 + L],
                                            K2[:, h, :], idf32[0:64, 0:64])
                    nc.vector.tensor_copy(
                        K2T[:], pt[0:64, :].rearrange("p (h s) -> p h s",
                                                      s=128)[:, :, 0:64])
                    K2Tb = nsp.tile([64, NHG, L], BF16, tag="K2Tb",
                                    name="K2Tb")
                    nc.vector.tensor_copy(K2Tb[:], K2T[:])
                    NSX["K2"], NSX["K2T"], NSX["K2Tb"] = K2, K2T, K2Tb

                def st_init():
                    K2, K2T = NSX["K2"], NSX["K2T"]
                    mxi = nsp.tile([64, NHG, 1], F32, tag="mxi", name="mxi")
                    cs = nsp.tile([64, NHG], F32, tag="cs", name="cs")
                    nc.vector.reduce_sum(cs[:], K2T[:], axis=AX.X)
                    for h in range(NHG):
                        pc = ps0w.tile([64, 512], F32, tag="ps0w",
                                       name=f"pc{h}")
                        nc.tensor.transpose(pc[0:1, 0:64], cs[:, h:h + 1],
                                            idf32[0:64, 0:64])
                        mx = nsp.tile([1, 1], F32, tag=f"mx{h}", name=f"mx{h}")
                        nc.vector.reduce_max(mx[:], pc[0:1, 0:64], axis=AX.X)
                        pb3 = ps0w.tile([64, 512], F32, tag="ps0w",
                                        name=f"pb3{h}")
                        nc.tensor.matmul(pb3[0:64, 0:1],
                                         lhsT=onesr[0:1, 0:64],
                                         rhs=mx[:], start=True, stop=True)
                        nc.vector.reciprocal(mxi[:, h, :], pb3[0:64, 0:1])
                    V = nsp.tile([64, NHG, L], BF16, tag="V", name="V0")
                    W = nsp.tile([64, NHG, L], BF16, tag="W", name="W0")
                    for h in range(NHG):
                        nc.vector.tensor_scalar_mul(V[:, h, :], K2T[:, h, :],
                                                    mxi[:, h, :])
                        nc.vector.tensor_scalar_mul(W[:, h, :], K2[:, h, :],
                                                    mxi[:, h, :])
                    NSX["V"], NSX["W"] = V, W

                ns_stages.append(st_K2)
                ns_stages.append(st_init)

            if P0LVL >= 6:
                # N-S iterations: per-head matmuls into per-head psum columns,
                # one batched DVE op per stage.
                def slots(pt):
                    return pt[:].rearrange("p (h s) -> p h s", s=128)[:, :, 0:64]

                NS = NSX
                NSD = [BF16] * (N_ITER - 1) + [F32]

                def ns_k2t(it):
                    return NS["K2Tb"] if NSD[it] == BF16 else NS["K2T"]

                def mk_stage(fn, it):
                    ns_stages.append(lambda fn=fn, it=it: fn(it))

                def st_A(it):
                    if it > 0 and NSD[it] != NSD[it - 1]:
                        Vf = nsp.tile([64, NHG, L], NSD[it], tag="Vf",
                                      name=f"Vf{it}")
                        Wf2 = nsp.tile([64, NHG, L], NSD[it], tag="Wf2",
                                       name=f"Wf2{it}")
                        nc.vector.tensor_copy(Vf[:], NS["V"][:])
                        nc.vector.tensor_copy(Wf2[:], NS["W"][:])
                        NS["V"], NS["W"] = Vf, Wf2
                    T1 = nsp.tile([64, NHG, L], NSD[it], tag="T1", name=f"T1_{it}")
                    KVT = nsp.tile([64, NHG, L], NSD[it], tag="KVT", name=f"KVT_{it}")
                    pAk = ps0w.tile([64, 512], F32, tag="ps0w", name=f"pAk{it}")
                    pAv = ps0w.tile([64, 512], F32, tag="ps0w", name=f"pAv{it}")
                    for h in range(NHG):
                        sl = slice(h * 128, h * 128 + L)
                        nc.tensor.matmul(pAk[:, sl], lhsT=ns_k2t(it)[:, h, :],
                                         rhs=NS["V"][:, h, :], start=True,
                                         stop=True, skip_group_check=True)
                        nc.tensor.matmul(pAv[:, sl], lhsT=NS["V"][:, h, :],
                                         rhs=ns_k2t(it)[:, h, :], start=True,
                                         stop=True, skip_group_check=True)
                    nc.vector.tensor_tensor(T1[:], nsc2[:, 0, :], slots(pAk),
                                            op=OP.subtract)
                    nc.scalar.copy(KVT[:], slots(pAv))
                    NS["T1"], NS["KVT"] = T1, KVT

                def st_B(it):
                    T2 = nsp.tile([64, NHG, L], NSD[it], tag="T2", name=f"T2_{it}")
                    pB = ps0w.tile([64, 512], F32, tag="ps0w", name=f"pB{it}")
                    for h in range(NHG):
                        nc.tensor.matmul(pB[:, h * 128:h * 128 + L],
                                         lhsT=NS["KVT"][:, h, :],
                                         rhs=NS["T1"][:, h, :], start=True,
                                         stop=True, skip_group_check=True)
                    nc.vector.tensor_tensor(T2[:], nsc2[:, 1, :], slots(pB),
                                            op=OP.subtract)
                    NS["T2"] = T2

                def st_C(it):
                    T3 = nsp.tile([64, NHG, L], NSD[it], tag="T3", name=f"T3_{it}")
                    pC = ps0w.tile([64, 512], F32, tag="ps0w", name=f"pC{it}")
                    for h in range(NHG):
                        nc.tensor.matmul(pC[:, h * 128:h * 128 + L],
                                         lhsT=NS["KVT"][:, h, :],
                                         rhs=NS["T2"][:, h, :], start=True,
                                         stop=True, skip_group_check=True)
                    nc.vector.scalar_tensor_tensor(T3[:], slots(pC), -0.25,
                                                   nsc2[:, 2, :],
                                                   op0=OP.mult, op1=OP.add)
                    NS["T3"] = T3

                def st_D(it):
                    last = it == N_ITER - 1
                    Wn = nsp.tile([64, NHG, L], NSD[min(it + 1, N_ITER - 1)], tag="W", name=f"W_{it + 1}")
                    pDw = ps0w.tile([64, 512], F32, tag="ps0w", name=f"pDw{it}")
                    pDv = None
                    if not last:
                        pDv = ps0w.tile([64, 512], F32, tag="ps0w",
                                        name=f"pDv{it}")
                    for h in range(NHG):
                        sl = slice(h * 128, h * 128 + L)
                        nc.tensor.matmul(pDw[:, sl], lhsT=NS["T3"][:, h, :],
                                         rhs=NS["W"][:, h, :], start=True,
                                         stop=True, skip_group_check=True)
                        if not last:
                            nc.tensor.matmul(pDv[:, sl], lhsT=NS["W"][:, h, :],
                                             rhs=NS["T3"][:, h, :], start=True,
                                             stop=True, skip_group_check=True)
                    nc.scalar.copy(Wn[:], slots(pDw))
                    if not last:
                        Vn = nsp.tile([64, NHG, L], NSD[it], tag="V",
                                      name=f"V_{it + 1}")
                        nc.vector.tensor_copy(Vn[:], slots(pDv))
                        NS["V"] = Vn
                    NS["W"] = Wn

                for it in range(N_ITER):
                    for fn in (st_A, st_B, st_C, st_D):
                        mk_stage(fn, it)

                def st_finish():
                    W_f2 = small.tile([64, NHG, L], F32, tag="Wf")
                    nc.vector.tensor_copy(W_f2[:], NS["W"][:])
                    W_holder["W"] = W_f2
                ns_stages.append(st_finish)

        if os.environ.get("K_NOA") == "1":
            with tc.tile_pool(name="zz2", bufs=1) as zz:
                z = zz.tile([128, E], BF16, tag="z")
                nc.vector.memset(z[:], 0.0)
                for c128 in range(64):
                    nc.sync.dma_start(out_d[bass.ts(c128, 128), :], z[:])
            return
        # ================= Phase A: v16 + kernel_3 =================
        # e3 kept resident; t1 runs as a standalone pass afterwards so the
        # phase-A psum budget leaves room for the interleaved N-S stages.
        e3sb = big.tile([128, 64, 256], BF16, tag="e3sb")
        with (
            tc.tile_pool(name="ps_v", bufs=2, space="PSUM") as ps_v,
            tc.tile_pool(name="ps_3", bufs=2, space="PSUM") as ps_3,
        ):
            for j in range(32):  # pairs of 128-seq chunks
                x8t = x8s[j // 8]
                xlot = xlos[j // 8]
                psv = ps_v.tile([128, 512], F32, tag="psv")
                ps3 = ps_3.tile([128, 512], F32, tag="ps3")
                for u in range(2):
                    c128 = 2 * j + u
                    sl = bass.ts(c128 % (SP // 128), 128)
                    usl = bass.ts(u, 256)
                    terms = ((x8t, wv16h), (x8t, wv16l), (xlot, wv16h))
                    i = 0
                    for xs, ws in terms:
                        for pr in range(2):
                            nc.tensor.matmul(
                                psv[:, usl],
                                lhsT=xs[:, 2 * pr:2 * pr + 2, sl],
                                rhs=ws[:, 2 * pr:2 * pr + 2, :],
                                start=(i == 0), stop=(i == 5),
                                perf_mode=PM.DoubleRow,
                                skip_group_check=True)
                            i += 1
                    for pr in range(2):
                        nc.tensor.matmul(
                            ps3[:, usl],
                            lhsT=x8t[:, 2 * pr:2 * pr + 2, sl],
                            rhs=M3T8[:, 2 * pr:2 * pr + 2, :],
                            start=(pr == 0), stop=(pr == 1),
                            perf_mode=PM.DoubleRow,
                            skip_group_check=True)
                if ns_stages:
                    ns_stages.pop(0)()
                vdst = vsb[:, 2 * j:2 * j + 2, :, 0:64]
                vsrc = psv[:].rearrange("p (c h d) -> p c h d", c=2, d=64)
                nc.vector.tensor_copy(vdst, vsrc)
                nc.scalar.activation(
                    e3sb[:, 2 * j:2 * j + 2, :].rearrange("p c m -> p (c m)"),
                    ps3[:], AF.Exp, scale=1.0 / M8)
            while ns_stages:
                ns_stages.pop(0)()
        ns_es.close()

        # ---- t1 accumulation pass ----
        with tc.tile_pool(name="ps_t1", bufs=4, space="PSUM") as ps_t1:
            t1ps = [ps_t1.tile([65, 64], F32, tag="t1", name=f"t1ps{h}")
                    for h in range(NHG)]
            for c128 in range(64):
                for h in range(NHG):
                    nc.tensor.matmul(
                        t1ps[h][:],
                        lhsT=vsb[:, c128, h, :],
                        rhs=e3sb[:, c128, h * 64:h * 64 + 64],
                        start=(c128 == 0), stop=(c128 == 63),
                        skip_group_check=True)
            # ======= Phase A': t1 -> t1n -> t2T -> m_sb =======
            with (
                tc.tile_pool(name="ps_m", bufs=2, space="PSUM") as ps_m,
                tc.tile_pool(name="ps_mE", bufs=2, space="PSUM") as ps_mE,
                tc.tile_pool(name="mp", bufs=2) as mp,
            ):
                W = W_holder["W"]
                t1n = mp.tile([64, NHG, 64], F32, tag="t1n", name="t1n")
                for h in range(NHG):
                    t1u = mp.tile([65, 64], F32, tag=f"t1u{h}", name=f"t1u{h}")
                    nc.vector.tensor_copy(t1u[:], t1ps[h][:])
                    ptt = ps_m.tile([64, 128], F32, tag="psm", name=f"ptt{h}")
                    nc.tensor.transpose(ptt[0:64, 0:65], t1u[:],
                                        idf32[0:65, 0:65])
                    d3i = mp.tile([64, 1], F32, tag=f"d3i{h}", name=f"d3i{h}")
                    nc.vector.reciprocal(d3i[:], ptt[0:64, 64:65])
                    nc.vector.tensor_scalar_mul(t1n[:, h, :],
                                                ptt[0:64, 0:64], d3i[:])
                nc.vector.tensor_tensor(t1n[:], t1n[:], bv16b[:],
                                        op=OP.add)
                t2T = mp.tile([64, NHG, 64], BF16, tag="t2T", name="t2T")
                pt2 = ps_mE.tile([64, 512], F32, tag="psmE", name="pt2")
                for h in range(NHG):
                    nc.tensor.matmul(pt2[:, h * 128:h * 128 + 64],
                                     lhsT=t1n[:, h, :], rhs=W[:, h, :],
                                     start=True, stop=True,
                                     skip_group_check=True)
                nc.vector.tensor_copy(
                    t2T[:], pt2[:].rearrange("p (h s) -> p h s",
                                             s=128)[:, :, 0:64])
                for h in range(NHG):
                    t, psl = h // 2, bass.ts(h % 2, 64)
                    pm_ = ps_mE.tile([64, 512], F32, tag="psmE", name=f"pm{h}")
                    nc.tensor.matmul(pm_[:], lhsT=t2T[:, h, :],
                                     rhs=wo16[:, h, :], start=True,
                                     stop=True)
                    nc.vector.tensor_copy(m_sb[psl, t, :], pm_[:])

        if os.environ.get("K_NOB") == "1":
            with tc.tile_pool(name="zz", bufs=1) as zz:
                z = zz.tile([128, E], BF16, tag="z")
                nc.vector.memset(z[:], 0.0)
                for c128 in range(64):
                    nc.sync.dma_start(out_d[bass.ts(c128, 128), :], z[:])
            return
        # ======= Phase B: kernel_1, normalize, output projection =======
        with (
            tc.tile_pool(name="ps_1", bufs=3, space="PSUM") as ps_1,
            tc.tile_pool(name="ps_r", bufs=2, space="PSUM") as ps_r,
            tc.tile_pool(name="ps_o", bufs=3, space="PSUM") as ps_o,
            tc.tile_pool(name="e1p", bufs=3) as e1p,
            tc.tile_pool(name="op", bufs=4) as op_,
        ):
            def emit_out(c, e1ns):
                # one batched store per chunk: 1 HWDGE descriptor-generation
                # instead of 4 (HWDGE is ~75% busy across phase B otherwise)
                osb = op_.tile([128, 4, 512], BF16, tag="osb")
                for s4 in range(4):
                    pso = ps_o.tile([128, 512], F32, tag="pso")
                    for t in range(2):
                        nc.tensor.matmul(pso[:],
                                         lhsT=e1ns[t][:, bass.ts(s4, 128)],
                                         rhs=m_sb[:, t, :],
                                         start=(t == 0), stop=(t == 1))
                    if s4 % 2 == 0:
                        nc.scalar.copy(osb[:, s4, :], pso[:])
                    else:
                        nc.vector.tensor_copy(osb[:, s4, :], pso[:])
                nc.sync.dma_start(
                    out_d[c * 512:(c + 1) * 512, :].rearrange(
                        "(s4 p) m -> p s4 m", p=128), osb[:])

            prev = None
            for c in range(16):
                x8t = x8s[c // 4]
                sl = bass.ts(c % 4, 512)
                e1ns = []
                pss = []
                for t in range(2):
                    ps1 = ps_1.tile([128, 512], F32, tag="ps1")
                    if USE_DR:
                        for pr in range(2):
                            nc.tensor.matmul(
                                ps1[:],
                                lhsT=M1T8[:, 2 * pr:2 * pr + 2, bass.ts(t, 128)],
                                rhs=x8t[:, 2 * pr:2 * pr + 2, sl],
                                start=(pr == 0), stop=(pr == 1),
                                perf_mode=PM.DoubleRow)
                    else:
                        for ko in range(4):
                            nc.tensor.matmul(
                                ps1[:], lhsT=M1T8[:, ko, bass.ts(t, 128)],
                                rhs=x8t[:, ko, sl],
                                start=(ko == 0), stop=(ko == 3))
                    pss.append(ps1)
                if prev is not None:
                    emit_out(*prev)
                for t in range(2):
                    ps1 = pss[t]
                    e1 = e1p.tile([128, 512], BF16, tag="e1")
                    nc.scalar.activation(e1[:], ps1[:], AF.Exp,
                                         bias=bias1[:, t, :], scale=1.0 / M8)
                    psr = ps_r.tile([128, 512], F32, tag="psr")
                    nc.tensor.matmul(psr[:], lhsT=blk1[:], rhs=e1[:],
                                     start=True, stop=True)
                    rbs = e1p.tile([128, 512], BF16, tag="rbs")
                    with nc.allow_low_precision(reason="softmax rowsum recip"):
                        nc.vector.reciprocal(rbs[:], psr[:])
                    e1n = e1p.tile([128, 512], BF16, tag="e1n")
                    eng = nc.vector if (t == 0 or not USE_GPSIMD) else nc.gpsimd
                    eng.tensor_tensor(e1n[:], e1[:], rbs[:], op=OP.mult)
                    e1ns.append(e1n)
                prev = (c, e1ns)
            emit_out(*prev)


def _prep_inputs(x, Wq, bq, Wk, bk, Wv, bv, Wo, bo):
    bf = ml_dtypes.bfloat16
    f8 = ml_dtypes.float8_e4m3
    x = np.asarray(x, dtype=np.float32)
    Wq = np.asarray(Wq, dtype=np.float32)
    Wk = np.asarray(Wk, dtype=np.float32)
    Wv = np.asarray(Wv, dtype=np.float32)
    Wo = np.asarray(Wo, dtype=np.float32)
    bq = np.asarray(bq, dtype=np.float32)
    bk = np.asarray(bk, dtype=np.float32)
    bv = np.asarray(bv, dtype=np.float32)

    eye64 = np.eye(64, dtype=np.float32)
    nsc2 = np.stack([
        np.tile(c * eye64, (1, 4))
        for c in (7.0, 15.0, 3.25)
    ], axis=1)  # [64, 3, 256]
    consts = {
        "nsc2": np.ascontiguousarray(nsc2.astype(np.float32)),
        "idf32": np.eye(128, dtype=np.float32),
        "blk1": np.ascontiguousarray(
            np.kron(np.eye(2), np.ones((64, 64))).astype(bf)),
        "onesr": np.ones((1, 128), dtype=np.float32),
    }

    per_batch = []
    for b in range(4):
        xT = np.ascontiguousarray(x[b].T)                      # [E, S] f32
        x8 = xT.astype(f8)
        xlo = (xT - x8.astype(np.float32)).astype(f8)
        xlT = np.ascontiguousarray(
            (x[b].reshape(64, 128, E).mean(axis=1).T * SCALE).astype(bf))
        per_batch.append((x8, xlo, xlT))

    in_maps = []
    for core in range(8):
        b, g = core // 2, core % 2
        hsl = slice(g * 256, (g + 1) * 256)
        x8, xlo, xlT = per_batch[b]
        wv16 = Wv[:, hsl] * 16.0
        wv16h = wv16.astype(f8)
        wv16l = (wv16 - wv16h.astype(np.float32)).astype(f8)
        # bv*16 broadcast over the 64 L partitions, [64, NHG, 64]
        bv16 = (bv[hsl] * 16.0).reshape(4, 64)
        bv16b = np.broadcast_to(bv16[None, :, :], (64, 4, 64)).astype(np.float32)
        bv16b = np.ascontiguousarray(bv16b)
        blob0 = np.concatenate(
            [xlT, Wq[:, hsl].astype(np.float32), Wk[:, hsl]], axis=1)
        in_maps.append({
            "x8": x8, "xlo": xlo,
            "blob0": np.ascontiguousarray(blob0).astype(bf),
            "blob1": np.ascontiguousarray(np.concatenate(
                [Wq[:, hsl].T, Wk[:, hsl].T,
                 (bq[hsl] * SCALE)[:, None], (bk[hsl] * SCALE)[:, None]],
                axis=1)).astype(bf),
            "wv16h": np.ascontiguousarray(wv16h),
            "wv16l": np.ascontiguousarray(wv16l),
            "wo16": np.ascontiguousarray(Wo[hsl, :] / 16.0).astype(bf),

            "bv16b": bv16b,
            **consts,
        })
    return in_maps


def run_on_device(in_maps, **kwargs):
    global _CACHED_NC
    if _CACHED_NC is None:
        _CACHED_NC = _build()
    return run_bass_kernel_spmd(_CACHED_NC, in_maps, core_ids=list(range(8)),
                                **kwargs)


def kernel(x, Wq, bq, Wk, bk, Wv, bv, Wo, bo):
    in_maps = _prep_inputs(x, Wq, bq, Wk, bk, Wv, bv, Wo, bo)
    res = run_on_device(in_maps)
    bo = np.asarray(bo, dtype=np.float32)
    out = np.empty((4, S, E), dtype=np.float32)
    for b in range(4):
        out[b] = (res.results[2 * b]["out"].astype(np.float32)
                  + res.results[2 * b + 1]["out"].astype(np.float32) + bo)
    return out



# revision 20
# speedup vs baseline: 1.1404x; 1.1404x over previous
"""Nystrom attention Trainium2 kernel (host-precomputed landmark branch).

Sharding: 8 cores = 4 batches x 2 head-groups (4 heads each). Each core
computes its (batch, head-group) slice; the host sums the two partial
output projections per batch (in f32) and adds bo.

The entire landmark branch is host-precomputed (it depends on x only
through the 64 segment means): q_land/k_land, kernel_2 softmax, the
Newton-Schulz pseudo-inverse (exact reference numerics in f32, including
the global-max init), M1 = SCALE^2 Wq k_land^T, M3 = SCALE^2 Wk q_land^T,
and bias1. The device then only runs the S-proportional work:

  Phase A  : v16 = x @ (Wv*16) (fp8 hi/lo DoubleRow, 3 terms),
             e3 = exp(x @ M3), t1[h] = [v16_h | 1]^T @ e3_h accumulated
             across chunks (lagged so PE never waits on the copies).
  Phase A' : t1n = t1/rowsum + bv*16; t2T via invK2^T; m = t2 @ Wo*(MSH/16)
             stored hi/lo fp8.
  Phase B  : software-pipelined per 512-seq chunk:
             logits1 (fp8 DR) -> exp (ACT, bf16) -> rowsum (block-ones
             matmul, bf16) -> recip (DVE) -> e1n = e1*rbs (fp8, x ES)
             -> out = e1n @ m8{h,lo} (fp8 DR) -> psum->bf16 copies
             (ACT/DVE/GP balanced) -> DMA.

Scales: M8 on M1/M3 (undone in exp), MSH=64 on m (fp8 normal range),
ES=32 on e1n via blkb=blk1/ES (fp8 normal range); out copy divides by
MSH*ES.
"""

import os
import numpy as np
import ml_dtypes

import concourse.bass as bass
import concourse.tile as tile
from concourse import bacc, mybir
from concourse.bass_utils import run_bass_kernel_spmd

BF16 = mybir.dt.bfloat16
F32 = mybir.dt.float32
FP8 = mybir.dt.float8e4
AF = mybir.ActivationFunctionType
AX = mybir.AxisListType
OP = mybir.AluOpType
PM = mybir.MatmulPerfMode

S = 8192        # sequence length
E = 512         # embedding dim
D = 64          # head dim
L = 64          # landmarks
NHG = 4         # heads per core (head group)
SCALE = 1.0 / np.sqrt(np.sqrt(D))
M8 = 64.0       # fp8 prescale on M1T/M3T (undone by exp scale)
MSH = 64.0      # fp8 prescale on m (via wo16); raw m rms ~0.005
ES = 32.0       # fp8 prescale on e1n (via blkb); e1n in [0, ES]
OSCALE = 1.0 / (MSH * ES)
SPLITS = [1024, 1024, 2048, 2048, 2048]   # x8/xlo DMA split sizes
NWARM = int(os.environ.get("K_NWARM", "11"))
T1LAG = 4

_CACHED_NC = None


def _build():
    nc = bacc.Bacc("TRN2", target_bir_lowering=False, debug=False, num_devices=8)

    dram = {}
    for name, shape, dt in [
        ("x8", [E, S], FP8),
        ("xlo", [E, S], FP8),
        ("m1t8", [E, 256], FP8),
        ("m3t8", [E, 256], FP8),
        ("wv16h", [E, 256], FP8),
        ("wv16l", [E, 256], FP8),
        ("wo16", [256, E], BF16),
        ("wt", [64, 256], F32),
        ("bias1", [128, 2], F32),
        ("bv16b", [64, 256], F32),
        ("blkb", [128, 128], BF16),
        ("idf32", [128, 128], F32),
    ]:
        dram[name] = nc.dram_tensor(name, shape, dt, kind="ExternalInput").ap()
    out_d = nc.dram_tensor("out", [S, E], BF16, kind="ExternalOutput").ap()

    with tile.TileContext(nc) as tc:
        _emit(nc, tc, dram, out_d)
    nc.compile()
    return nc


def _emit(nc, tc, dram, out_d):
    with (
        tc.tile_pool(name="const", bufs=1) as const,
        tc.tile_pool(name="big", bufs=1) as big,
    ):
        def load(name, shape, dt, pat=None, **kw):
            t = const.tile(shape, dt, tag=name)
            src = dram[name]
            if pat is not None:
                src = src.rearrange(pat, **kw)
            nc.sync.dma_start(t[:], src)
            return t

        # phase-A consts first so streaming can begin as soon as possible
        idf32 = load("idf32", [128, 128], F32)
        m3t8 = load("m3t8", [128, 4, 256], FP8, "(ko p) m -> p ko m", p=128)
        wv16h = load("wv16h", [128, 4, 256], FP8, "(ko p) m -> p ko m", p=128)
        wv16l = load("wv16l", [128, 4, 256], FP8, "(ko p) m -> p ko m", p=128)

        # x8/xlo splits; (tile, chunk128_base) per split
        x8s, xlos = [], []
        offs = []
        off = 0
        for i, ln in enumerate(SPLITS):
            offs.append(off)
            off += ln

        def load_split(i):
            ln, off = SPLITS[i], offs[i]
            ssl = slice(off, off + ln)
            t8 = big.tile([128, 4, ln], FP8, tag=f"x8_{i}")
            nc.sync.dma_start(
                t8[:], dram["x8"][:, ssl].rearrange("(ko p) s -> p ko s", p=128))
            tlo = big.tile([128, 4, ln], FP8, tag=f"xlo_{i}")
            nc.sync.dma_start(
                tlo[:], dram["xlo"][:, ssl].rearrange("(ko p) s -> p ko s", p=128))
            x8s.append(t8)
            xlos.append(tlo)

        def xslice(c128):
            """(x8_tile, xlo_tile, local 128-slice) for a 128-seq chunk."""
            s0 = c128 * 128
            for i, ln in enumerate(SPLITS):
                if s0 < offs[i] + ln:
                    lo = s0 - offs[i]
                    return x8s[i], xlos[i], slice(lo, lo + 128)
            raise AssertionError

        def xslice512(c):
            s0 = c * 512
            for i, ln in enumerate(SPLITS):
                if s0 < offs[i] + ln:
                    lo = s0 - offs[i]
                    return x8s[i], slice(lo, lo + 512)
            raise AssertionError

        load_split(0)
        load_split(1)
        # phase A'/B consts land while splits 0-1 are processed
        m1t8 = load("m1t8", [128, 4, 256], FP8, "(ko p) m -> p ko m", p=128)
        wt = load("wt", [64, 4, 64], F32, "p (h m) -> p h m", h=4)
        wo16 = load("wo16", [64, 4, E], BF16, "(h p) m -> p h m", p=64)
        bias1 = load("bias1", [128, 2], F32)
        bv16b = load("bv16b", [64, 4, 64], F32, "p (h m) -> p h m", h=4)
        blkb = load("blkb", [128, 128], BF16)
        for i in range(2, len(SPLITS)):
            load_split(i)

        # persistent activations / results
        vsb = big.tile([128, 64, NHG, 65], BF16, tag="v")  # (s | c, head, d+1)
        nc.vector.memset(vsb[:, :, :, 64:65], 1.0)
        e3sb = big.tile([128, 64, 256], BF16, tag="e3sb")
        m8h = const.tile([128, 2, E], FP8, tag="m8h")
        m8lo = const.tile([128, 2, E], FP8, tag="m8lo")

        # PE warm-up during the initial DMA window
        with tc.tile_pool(name="wu", bufs=1, space="PSUM") as wu:
            wps = wu.tile([128, 512], F32, tag="wps")
            for i in range(NWARM):
                nc.tensor.matmul(wps[:, 0:128], lhsT=idf32[:], rhs=idf32[:],
                                 start=True, stop=True, skip_group_check=True)

        # ============ Phase A: v16 + e3 + t1 accumulation ============
        with (
            tc.tile_pool(name="ps_v", bufs=2, space="PSUM") as ps_v,
            tc.tile_pool(name="ps_3", bufs=2, space="PSUM") as ps_3,
            tc.tile_pool(name="ps_t1", bufs=4, space="PSUM") as ps_t1,
        ):
            # one psum bank per head: start=True zeroes the whole 2KB bank,
            # so interleaved per-head accumulation groups must not share one
            t1ps = [ps_t1.tile([65, 64], F32, tag="t1ps", name=f"t1ps{h}")
                    for h in range(NHG)]

            def t1_step(j):
                for u in range(2):
                    c128 = 2 * j + u
                    for h in range(NHG):
                        nc.tensor.matmul(
                            t1ps[h][:],
                            lhsT=vsb[:, c128, h, :],
                            rhs=e3sb[:, c128, h * 64:h * 64 + 64],
                            start=(c128 == 0), stop=(c128 == 63),
                            skip_group_check=True)

            for j in range(32):  # pairs of 128-seq chunks
                psv = ps_v.tile([128, 512], F32, tag="psv")
                ps3 = ps_3.tile([128, 512], F32, tag="ps3")
                for u in range(2):
                    c128 = 2 * j + u
                    x8t, xlot, sl = xslice(c128)
                    usl = bass.ts(u, 256)
                    for pr in range(2):
                        nc.tensor.matmul(
                            ps3[:, usl],
                            lhsT=x8t[:, 2 * pr:2 * pr + 2, sl],
                            rhs=m3t8[:, 2 * pr:2 * pr + 2, :],
                            start=(pr == 0), stop=(pr == 1),
                            perf_mode=PM.DoubleRow,
                            skip_group_check=True)
                    i = 0
                    for xs, ws in ((x8t, wv16h), (x8t, wv16l), (xlot, wv16h)):
                        for pr in range(2):
                            nc.tensor.matmul(
                                psv[:, usl],
                                lhsT=xs[:, 2 * pr:2 * pr + 2, sl],
                                rhs=ws[:, 2 * pr:2 * pr + 2, :],
                                start=(i == 0), stop=(i == 5),
                                perf_mode=PM.DoubleRow,
                                skip_group_check=True)
                            i += 1
                if j >= T1LAG:
                    t1_step(j - T1LAG)
                vdst = vsb[:, 2 * j:2 * j + 2, :, 0:64]
                vsrc = psv[:].rearrange("p (c h d) -> p c h d", c=2, d=64)
                nc.vector.tensor_copy(vdst, vsrc)
                nc.scalar.activation(
                    e3sb[:, 2 * j:2 * j + 2, :].rearrange("p c m -> p (c m)"),
                    ps3[:], AF.Exp, scale=1.0 / M8)
            for j in range(32 - T1LAG, 32):
                t1_step(j)
            # t1 -> SBUF now so all phase-A psum can close before phase B
            t1u = const.tile([65, 256], F32, tag="t1u")
            for h in range(NHG):
                nc.vector.tensor_copy(t1u[:, h * 64:h * 64 + 64], t1ps[h][:])

        # ======= Phase B (pipelined) with A' overlapped at its head =======
        with (
            tc.tile_pool(name="ps_1", bufs=3, space="PSUM") as ps_1,
            tc.tile_pool(name="e1p", bufs=4) as e1p,
            tc.tile_pool(name="rp", bufs=3) as rp,
            tc.tile_pool(name="op", bufs=3) as op_,
            tc.tile_pool(name="mp", bufs=2) as mp,
        ):
            E1, E1N, PSR = {}, {}, {}

            def st_l1_exp(c):
                x8t, sl = xslice512(c)
                e1sb = e1p.tile([128, 2, 512], BF16, tag="e1")
                for t in range(2):
                    ps1 = ps_1.tile([128, 512], F32, tag="ps1")
                    for pr in range(2):
                        nc.tensor.matmul(
                            ps1[:],
                            lhsT=m1t8[:, 2 * pr:2 * pr + 2, bass.ts(t, 128)],
                            rhs=x8t[:, 2 * pr:2 * pr + 2, sl],
                            start=(pr == 0), stop=(pr == 1),
                            perf_mode=PM.DoubleRow)
                    nc.scalar.activation(e1sb[:, t, :], ps1[:], AF.Exp,
                                         bias=bias1[:, t:t + 1], scale=1.0 / M8)
                E1[c] = e1sb

            # ---- Phase A' chain; PE overlap from l1(0)/l1(1) above ----
            st_l1_exp(0)
            st_l1_exp(1)
            with tc.tile_pool(name="ps_m", bufs=2, space="PSUM") as ps_m:
                ptt = ps_m.tile([64, 512], F32, tag="psm", name="ptt")
                for h in range(NHG):
                    nc.tensor.transpose(ptt[:, h * 65:h * 65 + 65],
                                        t1u[:, h * 64:h * 64 + 64],
                                        idf32[0:65, 0:65])
                pttv = ptt[:, 0:260].rearrange("p (h m) -> p h m", m=65)
                d3i = mp.tile([64, 4, 1], F32, tag="d3i", name="d3i")
                nc.vector.reciprocal(d3i[:], pttv[:, :, 64:65])
                t1n = mp.tile([64, NHG, 64], F32, tag="t1n", name="t1n")
                for h in range(NHG):
                    nc.vector.tensor_scalar_mul(t1n[:, h, :],
                                                pttv[:, h, 0:64], d3i[:, h, :])
                nc.vector.tensor_tensor(t1n[:], t1n[:], bv16b[:], op=OP.add)
                # t2T[d, l] = sum_l' t1n[l', d] * invK2T[l', l]
                pt2 = ps_m.tile([64, 512], F32, tag="psm", name="pt2")
                for h in range(NHG):
                    nc.tensor.matmul(pt2[:, h * 64:h * 64 + 64],
                                     lhsT=t1n[:, h, :], rhs=wt[:, h, :],
                                     start=True, stop=True,
                                     skip_group_check=True)
                t2T = mp.tile([64, NHG, 64], BF16, tag="t2T", name="t2T")
                nc.vector.tensor_copy(
                    t2T[:], pt2[:, 0:256].rearrange("p (h m) -> p h m", h=4))
                for h in range(NHG):
                    t, psl = h // 2, bass.ts(h % 2, 64)
                    pm_ = ps_m.tile([64, 512], F32, tag="psm", name=f"pm{h}")
                    nc.tensor.matmul(pm_[:], lhsT=t2T[:, h, :],
                                     rhs=wo16[:, h, :], start=True,
                                     stop=True)
                    if h % 2 == 0:
                        nc.vector.tensor_copy(m8h[psl, t, :], pm_[:])
                    else:
                        nc.scalar.copy(m8h[psl, t, :], pm_[:])
                    nc.vector.tensor_tensor(m8lo[psl, t, :], pm_[:],
                                            m8h[psl, t, :], op=OP.subtract)

            if os.environ.get("K_DBG") == "1":
                dbg = mp.tile([128, 2, 512], BF16, tag="dbg", name="dbg")
                nc.vector.tensor_copy(dbg[:], m8h[:])
                for t in range(2):
                    nc.sync.dma_start(out_d[t * 128:(t + 1) * 128, :],
                                      dbg[:, t, :])
                dbg2 = mp.tile([128, 2, 512], BF16, tag="dbg", name="dbg2")
                nc.vector.tensor_copy(dbg2[:], m8lo[:])
                for t in range(2):
                    nc.sync.dma_start(out_d[256 + t * 128:256 + (t + 1) * 128, :],
                                      dbg2[:, t, :])
                dbg3 = mp.tile([65, 256], BF16, tag="dbg3", name="dbg3")
                nc.vector.tensor_copy(dbg3[:], t1u[:])
                nc.sync.dma_start(out_d[512:577, 0:256], dbg3[:])
                dbg4 = mp.tile([64, 256], BF16, tag="dbg4", name="dbg4")
                nc.vector.tensor_copy(
                    dbg4[:].rearrange("p (h m) -> p h m", h=4), t1n[:])
                nc.sync.dma_start(out_d[640:704, 0:256], dbg4[:])
                dbg5 = mp.tile([64, 256], BF16, tag="dbg5", name="dbg5")
                nc.vector.tensor_copy(
                    dbg5[:].rearrange("p (h m) -> p h m", h=4), t2T[:])
                nc.sync.dma_start(out_d[704:768, 0:256], dbg5[:])
                return

            def st_rsum(c):
                e1sb = E1[c]
                psrs = []
                for t in range(2):
                    psr = ps_r.tile([128, 512], F32, tag="psr")
                    nc.tensor.matmul(psr[:], lhsT=blkb[:],
                                     rhs=e1sb[:, t, :],
                                     start=True, stop=True,
                                     skip_group_check=True)
                    psrs.append(psr)
                PSR[c] = psrs

            def st_recip(c):
                psrs = PSR[c]
                rbs = rp.tile([128, 2, 512], BF16, tag="rbs")
                with nc.allow_low_precision(reason="softmax rowsum recip"):
                    for t in range(2):
                        nc.vector.reciprocal(rbs[:, t, :], psrs[t][:])
                PSR[c] = rbs

            def st_mult(c):
                e1sb, rbs = E1[c], PSR[c]
                e1n = e1p.tile([128, 2, 512], FP8, tag="e1n")
                for t in range(2):
                    nc.gpsimd.tensor_tensor(e1n[:, t, :], e1sb[:, t, :],
                                            rbs[:, t, :], op=OP.mult)
                E1N[c] = e1n
                del E1[c], PSR[c]

            def st_out(c):
                e1n = E1N[c]
                osb = op_.tile([128, 4, 512], BF16, tag="osb")
                for s4 in range(4):
                    pso = ps_o.tile([128, 512], F32, tag="pso")
                    for mm in (m8h, m8lo):
                        nc.tensor.matmul(pso[:],
                                         lhsT=e1n[:, 0:2, bass.ts(s4, 128)],
                                         rhs=mm[:, 0:2, :],
                                         start=(mm is m8h), stop=(mm is m8lo),
                                         perf_mode=PM.DoubleRow)
                    if s4 == 0:
                        nc.scalar.activation(osb[:, s4, :], pso[:],
                                             AF.Identity, scale=OSCALE)
                    elif s4 == 2:
                        nc.scalar.activation(osb[:, s4, 0:384], pso[:, 0:384],
                                             AF.Identity, scale=OSCALE)
                        nc.vector.tensor_scalar_mul(osb[:, s4, 384:512],
                                                    pso[:, 384:512], OSCALE)
                    else:
                        nc.vector.tensor_scalar_mul(osb[:, s4, :], pso[:],
                                                    OSCALE)
                    del pso
                nc.sync.dma_start(
                    out_d[c * 512:(c + 1) * 512, :].rearrange(
                        "(s4 p) m -> p s4 m", p=128), osb[:])
                del E1N[c]

            with (
                tc.tile_pool(name="ps_r", bufs=3, space="PSUM") as ps_r,
                tc.tile_pool(name="ps_o", bufs=2, space="PSUM") as ps_o,
            ):
                st_rsum(0)
                st_recip(0)
                for i in range(2, 19):
                    if i < 16:
                        st_l1_exp(i)
                    if 1 <= i - 1 < 16:
                        st_rsum(i - 1)
                        st_recip(i - 1)
                    if 0 <= i - 2 < 16:
                        st_mult(i - 2)
                    if 0 <= i - 3 < 16:
                        st_out(i - 3)


def _prep_inputs(x, Wq, bq, Wk, bk, Wv, bv, Wo, bo):
    bf = ml_dtypes.bfloat16
    f8 = ml_dtypes.float8_e4m3
    x = np.asarray(x, dtype=np.float32)
    Wq = np.asarray(Wq, dtype=np.float32)
    Wk = np.asarray(Wk, dtype=np.float32)
    Wv = np.asarray(Wv, dtype=np.float32)
    Wo = np.asarray(Wo, dtype=np.float32)
    bq = np.asarray(bq, dtype=np.float32)
    bk = np.asarray(bk, dtype=np.float32)
    bv = np.asarray(bv, dtype=np.float32)
    bs, NH = 4, 8
    S2 = SCALE * SCALE

    # ---- landmark branch (exact reference numerics, f32) ----
    x_land = x.reshape(bs, L, S // L, E).mean(axis=2)            # [bs, L, E]
    Q = x_land @ Wq + bq
    K = x_land @ Wk + bk
    Qh = Q.reshape(bs, L, NH, D).transpose(0, 2, 1, 3)           # [bs, h, L, d]
    Kh = K.reshape(bs, L, NH, D).transpose(0, 2, 1, 3)
    K2l = S2 * np.einsum('bhld,bhmd->bhlm', Qh, Kh)
    K2 = np.exp(K2l - K2l.max(-1, keepdims=True))
    K2 = (K2 / K2.sum(-1, keepdims=True)).astype(np.float32)
    I = np.eye(L, dtype=np.float32)
    V = K2.swapaxes(-1, -2) / np.max(np.sum(K2, axis=-2))
    for _ in range(6):
        KV = K2 @ V
        V = (0.25 * V) @ (13.0 * I - KV @ (15.0 * I - (KV @ (7.0 * I - KV))))
    invK2 = V

    Wqh = Wq.reshape(E, NH, D)
    Wkh = Wk.reshape(E, NH, D)
    M1 = S2 * np.einsum('ehd,bhld->behl', Wqh, Kh)               # [bs, E, h, L]
    M3 = S2 * np.einsum('ehd,bhld->behl', Wkh, Qh)
    bias1 = S2 * np.einsum('hd,bhld->bhl', bq.reshape(NH, D), Kh)

    blkb = (np.kron(np.eye(2, dtype=np.float32),
                    np.ones((64, 64), np.float32)) / ES).astype(bf)
    idf32 = np.eye(128, dtype=np.float32)

    per_batch = []
    for b in range(bs):
        xT = np.ascontiguousarray(x[b].T)
        x8 = xT.astype(f8)
        xlo = (xT - x8.astype(np.float32)).astype(f8)
        per_batch.append((x8, xlo))

    in_maps = []
    for core in range(8):
        b, g = core // 2, core % 2
        hsl = slice(g * 256, (g + 1) * 256)
        hh = slice(g * 4, (g + 1) * 4)
        x8, xlo = per_batch[b]
        wv16 = Wv[:, hsl] * 16.0
        wv16h = wv16.astype(f8)
        wv16l = (wv16 - wv16h.astype(np.float32)).astype(f8)
        bv16 = (bv[hsl] * 16.0).reshape(4, 64)
        bv16b = np.broadcast_to(bv16[None], (64, 4, 64)).astype(np.float32)
        m1c = np.ascontiguousarray(M1[b, :, hh, :].reshape(E, 256))
        m3c = np.ascontiguousarray(M3[b, :, hh, :].reshape(E, 256))
        b1 = bias1[b, hh, :].reshape(2, 2, 64)
        b1 = np.ascontiguousarray(b1.reshape(2, 128).T)
        wtc = np.ascontiguousarray(
            invK2[b, hh, :, :].transpose(2, 0, 1).reshape(64, 256))
        in_maps.append({
            "x8": x8, "xlo": xlo,
            "m1t8": (m1c * M8).astype(f8),
            "m3t8": (m3c * M8).astype(f8),
            "wv16h": np.ascontiguousarray(wv16h),
            "wv16l": np.ascontiguousarray(wv16l),
            "wo16": np.ascontiguousarray(Wo[hsl, :] * (MSH / 16.0)).astype(bf),
            "wt": wtc.astype(np.float32),
            "bias1": b1.astype(np.float32),
            "bv16b": np.ascontiguousarray(bv16b.reshape(64, 256)),
            "blkb": blkb,
            "idf32": idf32,
        })
    return in_maps


def run_on_device(in_maps, **kwargs):
    global _CACHED_NC
    if _CACHED_NC is None:
        _CACHED_NC = _build()
    return run_bass_kernel_spmd(_CACHED_NC, in_maps, core_ids=list(range(8)),
                                **kwargs)


def kernel(x, Wq, bq, Wk, bk, Wv, bv, Wo, bo):
    in_maps = _prep_inputs(x, Wq, bq, Wk, bk, Wv, bv, Wo, bo)
    res = run_on_device(in_maps)
    bo = np.asarray(bo, dtype=np.float32)
    out = np.empty((4, S, E), dtype=np.float32)
    for b in range(4):
        out[b] = (res.results[2 * b]["out"].astype(np.float32)
                  + res.results[2 * b + 1]["out"].astype(np.float32) + bo)
    return out


# revision 49
# speedup vs baseline: 1.4126x; 1.2387x over previous
"""Nystrom attention Trainium2 kernel (host-precomputed landmark branch).

Sharding: 8 cores = 4 batches x 2 head-groups (4 heads each). Each core
computes its (batch, head-group) slice; the host sums the two partial
output projections per batch (in f32) and adds bo.

The entire landmark branch is host-precomputed (it depends on x only
through the 64 segment means): q_land/k_land, kernel_2 softmax, the
Newton-Schulz pseudo-inverse (exact reference numerics in f32, including
the global-max init), M1 = SCALE^2 Wq k_land^T, M3 = SCALE^2 Wk q_land^T,
and bias1. The device then only runs the S-proportional work:

  Phase A  : v16 = x @ (Wv*16) (fp8 hi/lo DoubleRow, 3 terms),
             e3 = exp(x @ M3), t1[h] = [v16_h | 1]^T @ e3_h accumulated
             across chunks (lagged so PE never waits on the copies).
  Phase A' : t1n = t1/rowsum + bv*16; t2T via invK2^T; m = t2 @ Wo*(MSH/16)
             stored hi/lo fp8.
  Phase B  : software-pipelined per 512-seq chunk:
             logits1 (fp8 DR) -> exp (ACT, bf16) -> rowsum (block-ones
             matmul, bf16) -> recip (DVE) -> e1n = e1*rbs (fp8, x ES)
             -> out = e1n @ m8{h,lo} (fp8 DR) -> psum->bf16 copies
             (ACT/DVE/GP balanced) -> DMA.

Scales: M8 on M1/M3 (undone in exp), MSH=64 on m (fp8 normal range),
ES=32 on e1n via blkb=blk1/ES (fp8 normal range); out copy divides by
MSH*ES.
"""

import os
import numpy as np
import ml_dtypes

import concourse.bass as bass
import concourse.tile as tile
from concourse import bacc, mybir
from concourse.bass_utils import run_bass_kernel_spmd

BF16 = mybir.dt.bfloat16
F32 = mybir.dt.float32
FP8 = mybir.dt.float8e4
AF = mybir.ActivationFunctionType
AX = mybir.AxisListType
OP = mybir.AluOpType
PM = mybir.MatmulPerfMode

S = 8192        # sequence length
E = 512         # embedding dim
D = 64          # head dim
L = 64          # landmarks
NHG = 4         # heads per core (head group)
SCALE = 1.0 / np.sqrt(np.sqrt(D))
M8 = 64.0       # fp8 prescale on M1T/M3T (undone by exp scale)
MSH = 64.0      # fp8 prescale on m (via wo16); raw m rms ~0.005
ES = 32.0       # fp8 prescale on e1n (via blkb); e1n in [0, ES]
OSCALE = 1.0 / (MSH * ES)
SPLITS = [512, 512] + [1024] * 7              # x8/xlo DMA split sizes
NWARM = int(os.environ.get("K_NWARM", "8"))
T1LAG = 4

_CACHED_NC = None


def _build():
    nc = bacc.Bacc("TRN2", target_bir_lowering=False, debug=False, num_devices=8)

    dram = {}
    for name, shape, dt in [
        ("x8", [E, S], FP8),
        ("xlo", [E, S], FP8),
        ("m1t8", [E, 256], FP8),
        ("m3t8", [E, 256], FP8),
        ("wv16h", [E, 256], FP8),
        ("wv16l", [E, 256], FP8),
        ("wo16", [256, E], BF16),
        ("wt", [65, 256], F32),
        ("bias1", [128, 2], F32),
        ("bv16r", [1, 256], F32),
        ("blkb", [128, 128], BF16),
        ("idf32", [128, 128], F32),
    ]:
        dram[name] = nc.dram_tensor(name, shape, dt, kind="ExternalInput").ap()
    out_d = nc.dram_tensor("out", [S, E], BF16, kind="ExternalOutput").ap()

    with tile.TileContext(nc) as tc:
        _emit(nc, tc, dram, out_d)
    nc.compile()
    return nc


def _emit(nc, tc, dram, out_d):
    with (
        tc.tile_pool(name="const", bufs=1) as const,
        tc.tile_pool(name="big", bufs=1) as big,
    ):
        def load(name, shape, dt, pat=None, **kw):
            t = const.tile(shape, dt, tag=name)
            src = dram[name]
            if pat is not None:
                src = src.rearrange(pat, **kw)
            nc.sync.dma_start(t[:], src)
            return t

        # phase-A consts first so streaming can begin as soon as possible
        idf32 = load("idf32", [128, 128], F32)
        m3t8 = load("m3t8", [128, 4, 256], FP8, "(ko p) m -> p ko m", p=128)
        wv16h = load("wv16h", [128, 4, 256], FP8, "(ko p) m -> p ko m", p=128)
        wv16l = load("wv16l", [128, 4, 256], FP8, "(ko p) m -> p ko m", p=128)

        # x8/xlo splits; (tile, chunk128_base) per split
        x8s, xlos = [], []
        offs = []
        off = 0
        for i, ln in enumerate(SPLITS):
            offs.append(off)
            off += ln

        def load_split(i):
            ln, off = SPLITS[i], offs[i]
            ssl = slice(off, off + ln)
            t8 = big.tile([128, 4, ln], FP8, tag=f"x8_{i}")
            nc.sync.dma_start(
                t8[:], dram["x8"][:, ssl].rearrange("(ko p) s -> p ko s", p=128))
            tlo = big.tile([128, 4, ln], FP8, tag=f"xlo_{i}")
            nc.sync.dma_start(
                tlo[:], dram["xlo"][:, ssl].rearrange("(ko p) s -> p ko s", p=128))
            x8s.append(t8)
            xlos.append(tlo)

        def xslice(c128):
            """(x8_tile, xlo_tile, local 128-slice) for a 128-seq chunk."""
            s0 = c128 * 128
            for i, ln in enumerate(SPLITS):
                if s0 < offs[i] + ln:
                    lo = s0 - offs[i]
                    return x8s[i], xlos[i], slice(lo, lo + 128)
            raise AssertionError

        def xslice512(c):
            s0 = c * 512
            for i, ln in enumerate(SPLITS):
                if s0 < offs[i] + ln:
                    lo = s0 - offs[i]
                    return x8s[i], slice(lo, lo + 512)
            raise AssertionError

        load_split(0)
        load_split(1)
        load_split(2)
        m1t8 = load("m1t8", [128, 4, 256], FP8, "(ko p) m -> p ko m", p=128)
        bias1 = load("bias1", [128, 2], F32)
        blkb = load("blkb", [128, 128], BF16)
        load_split(3)
        # remaining A' consts land while splits 0-3 are processed
        wt65 = load("wt", [65, 4, 64], F32, "p (h m) -> p h m", h=4)
        wo16 = load("wo16", [64, 4, E], BF16, "(h p) m -> p h m", p=64)
        t1n65 = const.tile([65, 4, 64], F32, tag="t1n65")
        nc.sync.dma_start(t1n65[64:65, :, :],
                          dram["bv16r"].rearrange("o (h m) -> o h m", h=4))
        for i in range(4, len(SPLITS)):
            load_split(i)

        # persistent activations / results
        vsb = big.tile([128, 64, NHG, 65], BF16, tag="v")  # (s | c, head, d+1)
        nc.vector.memset(vsb[:, :, :, 64:65], 1.0)
        e3sb = big.tile([128, 64, 256], BF16, tag="e3sb")
        m8h = const.tile([128, 2, E], FP8, tag="m8h")
        e1nall = big.tile([128, 16, 2, 512], FP8, tag="e1nall")

        # PE warm-up during the initial DMA window; memset-seeded operand so
        # the first matmul needs no DMA to have landed
        with tc.tile_pool(name="wu", bufs=1, space="PSUM") as wu:
            wsb = const.tile([128, 512], BF16, tag="wsb")
            nc.vector.memset(wsb[:], 1.0)
            wps = wu.tile([128, 512], F32, tag="wps")
            for i in range(NWARM):
                nc.tensor.matmul(wps[:], lhsT=wsb[:, 0:128], rhs=wsb[:],
                                 start=True, stop=True, skip_group_check=True)

        # ==== Phase A (fused with kernel_1 B1 pipeline): v16 + e3 + t1 ====
        with (
            tc.tile_pool(name="ps_v", bufs=2, space="PSUM") as ps_v,
            tc.tile_pool(name="ps_3", bufs=2, space="PSUM") as ps_3,
            tc.tile_pool(name="ps_t1", bufs=1, space="PSUM") as ps_t1,
            tc.tile_pool(name="ps_1", bufs=3, space="PSUM") as ps_1,
            tc.tile_pool(name="e1p", bufs=3) as e1p,
            tc.tile_pool(name="rp", bufs=3) as rp,
        ):
            ps_r = ps_1  # rowsum psums share the logits1 pool (disjoint use)
            # t1 computed transposed: out [l, (v|1)] so no PE transposes are
            # needed in phase A'. ONE accumulation group for all heads x
            # chunks: start=True zeroes the whole 2KB bank, so only the very
            # first matmul starts; later first-writes to untouched regions
            # overwrite-and-mark via has_written, then accumulate.
            t1ps = ps_t1.tile([64, 4, 65], F32, tag="t1ps")

            def t1_step(j):
                for u in range(2):
                    c128 = 2 * j + u
                    for h in range(NHG):
                        nc.tensor.matmul(
                            t1ps[:, h, :],
                            lhsT=e3sb[:, c128, h * 64:h * 64 + 64],
                            rhs=vsb[:, c128, h, :],
                            start=(c128 == 0 and h == 0),
                            stop=(c128 == 63 and h == 3),
                            skip_group_check=True)

            # ---- B1 pipeline stages (interleaved into the pair loop) ----
            E1, RB = {}, {}

            def st_l1_exp(c):
                x8t, sl = xslice512(c)
                e1sb = e1p.tile([128, 2, 512], BF16, tag="e1")
                for t in range(2):
                    ps1 = ps_1.tile([128, 512], F32, tag="ps1")
                    for pr in range(2):
                        nc.tensor.matmul(
                            ps1[:],
                            lhsT=m1t8[:, 2 * pr:2 * pr + 2, bass.ts(t, 128)],
                            rhs=x8t[:, 2 * pr:2 * pr + 2, sl],
                            start=(pr == 0), stop=(pr == 1),
                            perf_mode=PM.DoubleRow)
                    nc.scalar.activation(e1sb[:, t, :], ps1[:], AF.Exp,
                                         bias=bias1[:, t:t + 1], scale=1.0 / M8)
                E1[c] = e1sb

            def st_rsum_recip(c):
                e1sb = E1[c]
                rbs = rp.tile([128, 2, 512], BF16, tag="rbs")
                for t in range(2):
                    psr = ps_r.tile([128, 512], F32, tag="ps1")
                    nc.tensor.matmul(psr[:], lhsT=blkb[:],
                                     rhs=e1sb[:, t, :],
                                     start=True, stop=True,
                                     skip_group_check=True)
                    with nc.allow_low_precision(reason="softmax rowsum recip"):
                        nc.vector.reciprocal(rbs[:, t, :], psr[:])
                RB[c] = rbs

            def st_mult(c):
                e1sb, rbs = E1[c], RB[c]
                for t in range(2):
                    nc.gpsimd.tensor_tensor(e1nall[:, c, t, :], e1sb[:, t, :],
                                            rbs[:, t, :], op=OP.mult)
                del E1[c], RB[c]

            def b1_steps(j):
                # l1(c) at pair 2c+2, rsum at 2c+3, mult at 2c+4
                if j >= 2 and j % 2 == 0 and (j - 2) // 2 < 16:
                    st_l1_exp((j - 2) // 2)
                if j >= 3 and j % 2 == 1 and (j - 3) // 2 < 16:
                    st_rsum_recip((j - 3) // 2)
                if j >= 4 and j % 2 == 0 and (j - 4) // 2 < 16:
                    st_mult((j - 4) // 2)

            for j in range(32):  # pairs of 128-seq chunks
                psv = ps_v.tile([128, 512], F32, tag="psv")
                ps3 = ps_3.tile([128, 512], F32, tag="ps3")
                for u in range(2):
                    c128 = 2 * j + u
                    x8t, xlot, sl = xslice(c128)
                    usl = bass.ts(u, 256)
                    for pr in range(2):
                        nc.tensor.matmul(
                            ps3[:, usl],
                            lhsT=x8t[:, 2 * pr:2 * pr + 2, sl],
                            rhs=m3t8[:, 2 * pr:2 * pr + 2, :],
                            start=(pr == 0), stop=(pr == 1),
                            perf_mode=PM.DoubleRow,
                            skip_group_check=True)
                    if u == 0:
                        b1_steps(j)  # B1 PE work before v16 so the single
                        # psv buffer's copy has time to drain
                    i = 0
                    for xs, ws in ((x8t, wv16h), (x8t, wv16l), (xlot, wv16h)):
                        for pr in range(2):
                            nc.tensor.matmul(
                                psv[:, usl],
                                lhsT=xs[:, 2 * pr:2 * pr + 2, sl],
                                rhs=ws[:, 2 * pr:2 * pr + 2, :],
                                start=(i == 0), stop=(i == 5),
                                perf_mode=PM.DoubleRow,
                                skip_group_check=True)
                            i += 1
                if j >= T1LAG:
                    t1_step(j - T1LAG)
                vdst = vsb[:, 2 * j:2 * j + 2, :, 0:64]
                vsrc = psv[:].rearrange("p (c h d) -> p c h d", c=2, d=64)
                nc.vector.tensor_copy(vdst, vsrc)
                nc.scalar.activation(
                    e3sb[:, 2 * j:2 * j + 2, :].rearrange("p c m -> p (c m)"),
                    ps3[:], AF.Exp, scale=1.0 / M8)
            # tail: finish t1 first, then fill the A' latency with the
            # remaining B1 stages (c14 mult, c15 full)
            for j in range(32, 36):
                t1_step(j - T1LAG)
            st_l1_exp(15)
            # ---- Phase A' (inside phase-A scope: reads t1ps psum) ----
            d3i = const.tile([64, 4, 1], F32, tag="d3i")
            nc.vector.reciprocal(d3i[:], t1ps[:, :, 64:65])
            st_mult(14)
            for h in range(NHG):
                nc.vector.tensor_scalar_mul(t1n65[0:64, h, :],
                                            t1ps[:, h, 0:64], d3i[:, h, :])
            st_rsum_recip(15)
            # t2T[d, l] = sum_l' t1n65[l', d] wt65[l', l]  (row 64 folds bv)
            t2T = const.tile([64, NHG, 64], BF16, tag="t2T")
            pt2 = ps_1.tile([128, 512], F32, tag="ps1", name="pt2")
            for h in range(NHG):
                nc.tensor.matmul(pt2[0:64, h * 64:h * 64 + 64],
                                 lhsT=t1n65[:, h, :], rhs=wt65[:, h, :],
                                 start=(h == 0), stop=(h == 3),
                                 skip_group_check=True)
            nc.vector.tensor_copy(
                t2T[:], pt2[0:64, 0:256].rearrange("p (h m) -> p h m", h=4))
            st_mult(15)
            for h in range(NHG):
                t, psl = h // 2, bass.ts(h % 2, 64)
                pm_ = ps_1.tile([128, 512], F32, tag="ps1", name=f"pm{h}")
                nc.tensor.matmul(pm_[0:64, :], lhsT=t2T[:, h, :],
                                 rhs=wo16[:, h, :], start=True, stop=True,
                                 skip_group_check=True)
                if h % 2 == 0:
                    nc.vector.tensor_copy(m8h[psl, t, :], pm_[0:64, :])
                else:
                    nc.scalar.copy(m8h[psl, t, :], pm_[0:64, :])

        # ======= B2: output projection, DMA-paced =======
        with tc.tile_pool(name="op", bufs=4) as op_:
            with tc.tile_pool(name="ps_o", bufs=4, space="PSUM") as ps_o:
                for c in range(16):
                    osb = op_.tile([128, 4, 512], BF16, tag="osb")
                    for s4 in range(4):
                        pso = ps_o.tile([128, 512], F32, tag="pso")
                        nc.tensor.matmul(
                            pso[:],
                            lhsT=e1nall[:, c, 0:2, bass.ts(s4, 128)],
                            rhs=m8h[:, 0:2, :],
                            start=True, stop=True,
                            perf_mode=PM.DoubleRow)
                        if s4 % 2 == 0:
                            nc.scalar.activation(osb[:, s4, :], pso[:],
                                                 AF.Identity, scale=OSCALE)
                        else:
                            nc.vector.tensor_scalar_mul(osb[:, s4, :], pso[:],
                                                        OSCALE)
                    nc.sync.dma_start(
                        out_d[c * 512:(c + 1) * 512, :].rearrange(
                            "(s4 p) m -> p s4 m", p=128), osb[:])


def _prep_inputs(x, Wq, bq, Wk, bk, Wv, bv, Wo, bo):
    bf = ml_dtypes.bfloat16
    f8 = ml_dtypes.float8_e4m3
    x = np.asarray(x, dtype=np.float32)
    Wq = np.asarray(Wq, dtype=np.float32)
    Wk = np.asarray(Wk, dtype=np.float32)
    Wv = np.asarray(Wv, dtype=np.float32)
    Wo = np.asarray(Wo, dtype=np.float32)
    bq = np.asarray(bq, dtype=np.float32)
    bk = np.asarray(bk, dtype=np.float32)
    bv = np.asarray(bv, dtype=np.float32)
    bs, NH = 4, 8
    S2 = SCALE * SCALE

    # ---- landmark branch (exact reference numerics, f32) ----
    x_land = x.reshape(bs, L, S // L, E).mean(axis=2)            # [bs, L, E]
    Q = x_land @ Wq + bq
    K = x_land @ Wk + bk
    Qh = Q.reshape(bs, L, NH, D).transpose(0, 2, 1, 3)           # [bs, h, L, d]
    Kh = K.reshape(bs, L, NH, D).transpose(0, 2, 1, 3)
    K2l = S2 * np.einsum('bhld,bhmd->bhlm', Qh, Kh)
    K2 = np.exp(K2l - K2l.max(-1, keepdims=True))
    K2 = (K2 / K2.sum(-1, keepdims=True)).astype(np.float32)
    I = np.eye(L, dtype=np.float32)
    V = K2.swapaxes(-1, -2) / np.max(np.sum(K2, axis=-2))
    for _ in range(6):
        KV = K2 @ V
        V = (0.25 * V) @ (13.0 * I - KV @ (15.0 * I - (KV @ (7.0 * I - KV))))
    invK2 = V

    Wqh = Wq.reshape(E, NH, D)
    Wkh = Wk.reshape(E, NH, D)
    M1 = S2 * np.einsum('ehd,bhld->behl', Wqh, Kh)               # [bs, E, h, L]
    M3 = S2 * np.einsum('ehd,bhld->behl', Wkh, Qh)
    bias1 = S2 * np.einsum('hd,bhld->bhl', bq.reshape(NH, D), Kh)

    blkb = (np.kron(np.eye(2, dtype=np.float32),
                    np.ones((64, 64), np.float32)) / ES).astype(bf)
    idf32 = np.eye(128, dtype=np.float32)

    per_batch = []
    for b in range(bs):
        xT = np.ascontiguousarray(x[b].T)
        x8 = xT.astype(f8)
        xlo = (xT - x8.astype(np.float32)).astype(f8)
        per_batch.append((x8, xlo))

    in_maps = []
    for core in range(8):
        b, g = core // 2, core % 2
        hsl = slice(g * 256, (g + 1) * 256)
        hh = slice(g * 4, (g + 1) * 4)
        x8, xlo = per_batch[b]
        wv16 = Wv[:, hsl] * 16.0
        wv16h = wv16.astype(f8)
        wv16l = (wv16 - wv16h.astype(np.float32)).astype(f8)
        bv16 = (bv[hsl] * 16.0).reshape(4, 64)
        bv16b = np.broadcast_to(bv16[None], (64, 4, 64)).astype(np.float32)
        m1c = np.ascontiguousarray(M1[b, :, hh, :].reshape(E, 256))
        m3c = np.ascontiguousarray(M3[b, :, hh, :].reshape(E, 256))
        b1 = bias1[b, hh, :].reshape(2, 2, 64)
        b1 = np.ascontiguousarray(b1.reshape(2, 128).T)
        # wt65 rows 0-63: invK2^T per head; row 64: invK2 row-sums (pairs
        # with the constant bv row of t1n65 to fold the bv*16 bias in)
        wtc = np.concatenate(
            [invK2[b, hh, :, :].transpose(2, 0, 1),
             invK2[b, hh, :, :].sum(axis=2)[None]], axis=0)   # [65, 4, 64]
        in_maps.append({
            "x8": x8, "xlo": xlo,
            "m1t8": (m1c * M8).astype(f8),
            "m3t8": (m3c * M8).astype(f8),
            "wv16h": np.ascontiguousarray(wv16h),
            "wv16l": np.ascontiguousarray(wv16l),
            "wo16": np.ascontiguousarray(Wo[hsl, :] * (MSH / 16.0)).astype(bf),
            "wt": np.ascontiguousarray(wtc.reshape(65, 256)).astype(np.float32),
            "bias1": b1.astype(np.float32),
            "bv16r": np.ascontiguousarray(bv16.reshape(1, 256)),
            "blkb": blkb,
            "idf32": idf32,
        })
    return in_maps


def run_on_device(in_maps, **kwargs):
    global _CACHED_NC
    if _CACHED_NC is None:
        _CACHED_NC = _build()
    return run_bass_kernel_spmd(_CACHED_NC, in_maps, core_ids=list(range(8)),
                                **kwargs)


def kernel(x, Wq, bq, Wk, bk, Wv, bv, Wo, bo):
    in_maps = _prep_inputs(x, Wq, bq, Wk, bk, Wv, bv, Wo, bo)
    res = run_on_device(in_maps)
    bo = np.asarray(bo, dtype=np.float32)
    out = np.empty((4, S, E), dtype=np.float32)
    for b in range(4):
        out[b] = (res.results[2 * b]["out"].astype(np.float32)
                  + res.results[2 * b + 1]["out"].astype(np.float32) + bo)
    return out
